# revision 9
# baseline (speedup 1.0000x reference)
"""Trainium2 Bass kernel for CurveChannel: piecewise-linear per-channel curve
+ 1x1 conv (C->1) + hardtanh(0,1).

out[b,0,h,w] = clip( sum_{p,c} W[p,c] * relu(x[b,c,h,w] - shift[p,c]) + conv_b,
                     0, 1 )         where W[p,c] = slopes[p,c] * conv_w[c]

Sharding: pure data parallel over batch (8 images -> 8 cores). Params are tiny
and get folded host-side into per-(p,c) weights; zero-weight terms contribute
exactly 0 and are skipped. For W>0 the scale/bias fold into one ScalarE
activation: W*relu(x - s) == relu(W*x - W*s).
"""

import os

import numpy as np

import concourse.bacc as bacc
import concourse.bass as bass
import concourse.mybir as mybir
import concourse.tile as tile
from concourse.bass_utils import run_bass_kernel_spmd

N_CORES = 8
C_IN = 3
H = 512
W_IMG = 512
P = 128                      # SBUF partitions
SPATIAL = H * W_IMG          # 262144
FREE = SPATIAL // P          # 2048 fp32 per partition per channel
CHUNK = 512                  # free-dim tile size
N_CHUNKS = FREE // CHUNK

F32 = mybir.dt.float32

LAST_RESULTS = None          # BassKernelResults of the most recent run (for test.py)


def _build_nc(terms, bias):
    """terms: list of (channel, weight, shift) with weight != 0."""
    nc = bacc.Bacc(trn_type="TRN2", debug=False)
    x_t = nc.dram_tensor("x", [C_IN, P, FREE], F32, kind="ExternalInput")
    out_t = nc.dram_tensor("out", [P, FREE], F32, kind="ExternalOutput")

    # Split terms by weight sign. Both signs lower to a pure-ACT slice write:
    #   w > 0:  w*relu(x-s) ==  relu(w*x - w*s)
    #   w < 0:  w*relu(x-s) == -relu(-w*x + w*s)   (subtracted via second reduce)
    pos = [(c, w, s) for c, w, s in terms if w > 0]
    neg = [(c, w, s) for c, w, s in terms if w < 0]
    ordered = pos + neg
    used_channels = sorted({c for c, _, _ in terms})
    cidx = {c: i for i, c in enumerate(used_channels)}
    nch = len(used_channels)
    nt = len(ordered)

    # Design rule: every instruction must need at most ONE semaphore wait
    # (several ISA structs have a single sync-wait slot), and no DMA sem lane
    # may serve two DMAs (the second picks up a lane-FIFO wait on top of its
    # data wait). Hence: one combined in-DMA per chunk, all wide-tile slices
    # written by ACT only, and out-DMAs on the SWDGE lanes.
    with tile.TileContext(nc) as tc:
        with (
            tc.tile_pool(name="xin", bufs=N_CHUNKS) as xpool,
            tc.tile_pool(name="work", bufs=N_CHUNKS) as wpool,
            tc.tile_pool(name="out", bufs=1) as opool,
        ):
            # one wide result tile; per-chunk clips write slices, and a single
            # 1 MiB DMA stores it (fewest DMA sem lanes -> the tail drain stays
            # under the CTRL struct's sync-wait capacity; also best DMA shape)
            res_w = opool.tile([P, FREE], F32, tag="res")
            for j in range(N_CHUNKS):
                res = res_w[:, bass.ts(j, CHUNK)]
                if nt == 0:
                    nc.vector.memset(res, float(np.clip(bias, 0.0, 1.0)))
                    continue

                xt = xpool.tile([P, nch * CHUNK], F32, tag="x")
                src = x_t[used_channels[0]:used_channels[-1] + 1, :,
                          bass.ts(j, CHUNK)] if nch == C_IN else None
                if src is not None:
                    nc.sync.dma_start(
                        out=xt[:], in_=src.rearrange("c p f -> p c f")
                    )
                else:
                    for c in used_channels:
                        nc.sync.dma_start(
                            out=xt[:, bass.ts(cidx[c], CHUNK)],
                            in_=x_t[c, :, bass.ts(j, CHUNK)],
                        )

                wide = wpool.tile([P, nt * CHUNK], F32, tag="wide")
                for i, (c, w, s) in enumerate(ordered):
                    sl = wide[:, bass.ts(i, CHUNK)]
                    xs = xt[:, bass.ts(cidx[c], CHUNK)]
                    if w > 0:
                        nc.scalar.activation(
                            sl, xs, mybir.ActivationFunctionType.Relu,
                            bias=-w * s, scale=w,
                        )
                    else:
                        nc.scalar.activation(
                            sl, xs, mybir.ActivationFunctionType.Relu,
                            bias=w * s, scale=-w,
                        )

                def reduce_slices(lo, hi, tag):
                    n = hi - lo
                    dst = wpool.tile([P, CHUNK], F32, tag=tag)
                    if n == 1:
                        return wide[:, bass.ts(lo, CHUNK)]
                    v = wide[:, lo * CHUNK:hi * CHUNK].rearrange(
                        "p (c f) -> p f c", c=n
                    )
                    nc.vector.tensor_reduce(
                        dst[:], v, axis=mybir.AxisListType.X,
                        op=mybir.AluOpType.add,
                    )
                    return dst[:]

                if neg and pos:
                    rp = reduce_slices(0, len(pos), "redp")
                    rn = reduce_slices(len(pos), nt, "redn")
                    comb = wpool.tile([P, CHUNK], F32, tag="comb")
                    nc.vector.tensor_sub(comb[:], rp, rn)
                    comb = comb[:]
                elif pos:
                    comb = reduce_slices(0, len(pos), "redp")
                else:
                    rn = reduce_slices(0, nt, "redn")
                    comb = wpool.tile([P, CHUNK], F32, tag="comb")
                    nc.vector.tensor_scalar_mul(comb[:], rn, -1.0)
                    comb = comb[:]

                if bias != 0.0:
                    nc.vector.tensor_scalar(
                        res, comb, bias, 0.0,
                        mybir.AluOpType.add, mybir.AluOpType.max,
                    )
                    nc.vector.tensor_scalar_min(res, res, 1.0)
                else:
                    nc.vector.tensor_scalar(
                        res, comb, 0.0, 1.0,
                        mybir.AluOpType.max, mybir.AluOpType.min,
                    )
            nc.sync.dma_start(out=out_t[:, :], in_=res_w[:])
    nc.compile()
    return nc


_NC_CACHE = {}


def kernel(x, shift, slopes, conv_w, conv_b):
    global LAST_RESULTS
    x = np.ascontiguousarray(np.asarray(x, dtype=np.float32))
    shift = np.asarray(shift, dtype=np.float32)
    slopes = np.asarray(slopes, dtype=np.float32)
    conv_w = np.asarray(conv_w, dtype=np.float32)
    conv_b = np.asarray(conv_b, dtype=np.float32)

    B = x.shape[0]
    assert x.shape == (N_CORES, C_IN, H, W_IMG), x.shape

    wmat = slopes * conv_w[None, :]                      # (npts, C)
    npts = wmat.shape[0]
    terms = tuple(
        (c, float(wmat[p, c]), float(shift[p, c]))
        for p in range(npts) for c in range(C_IN)
        if wmat[p, c] != 0.0
    )
    bias = float(conv_b.reshape(-1)[0])

    key = (terms, bias)
    nc = _NC_CACHE.get(key)
    if nc is None:
        nc = _build_nc(terms, bias)
        _NC_CACHE[key] = nc

    xs = x.reshape(B, C_IN, P, FREE)
    in_maps = [{"x": xs[i]} for i in range(N_CORES)]
    trace = bool(int(os.environ.get("KERNEL_TRACE", "0")))
    LAST_RESULTS = run_bass_kernel_spmd(
        nc, in_maps, list(range(N_CORES)), trace=trace
    )
    out = np.stack(
        [LAST_RESULTS.results[i]["out"].reshape(1, H, W_IMG) for i in range(N_CORES)],
        axis=0,
    )
    return out.astype(np.float32, copy=False)


# revision 11
# speedup vs baseline: 798529.1362x; 798529.1362x over previous
"""Trainium2 Bass kernel for CurveChannel: piecewise-linear per-channel curve
+ 1x1 conv (C->1) + hardtanh(0,1).

out[b,0,h,w] = clip( sum_{p,c} W[p,c] * relu(x[b,c,h,w] - shift[p,c]) + conv_b,
                     0, 1 )         where W[p,c] = slopes[p,c] * conv_w[c]

Sharding: pure data parallel over batch (8 images -> 8 cores). Params are tiny
and get folded host-side into per-(p,c) weights; zero-weight terms contribute
exactly 0 and are skipped. For W>0 the scale/bias fold into one ScalarE
activation: W*relu(x - s) == relu(W*x - W*s).
"""

import os

import numpy as np

import concourse.bacc as bacc
import concourse.bass as bass
import concourse.mybir as mybir
import concourse.tile as tile
from concourse.bass_utils import run_bass_kernel_spmd

N_CORES = 8
C_IN = 3
H = 512
W_IMG = 512
P = 128                      # SBUF partitions
SPATIAL = H * W_IMG          # 262144
FREE = SPATIAL // P          # 2048 fp32 per partition per channel
CHUNK = 512                  # free-dim tile size
N_CHUNKS = FREE // CHUNK

F32 = mybir.dt.float32

LAST_RESULTS = None          # BassKernelResults of the most recent run (for test.py)


def _build_nc(terms, bias, reps=1):
    """terms: list of (channel, weight, shift) with weight != 0.

    reps > 1 unrolls the whole pass multiple times over the same data --
    only used for benchmarking (marginal time per pass = device time with
    host/RPC constants cancelled).
    """
    nc = bacc.Bacc(trn_type="TRN2", debug=False)
    x_t = nc.dram_tensor("x", [C_IN, P, FREE], F32, kind="ExternalInput")
    out_t = nc.dram_tensor("out", [P, FREE], F32, kind="ExternalOutput")

    # Split terms by weight sign. Both signs lower to a pure-ACT slice write:
    #   w > 0:  w*relu(x-s) ==  relu(w*x - w*s)
    #   w < 0:  w*relu(x-s) == -relu(-w*x + w*s)   (subtracted via second reduce)
    pos = [(c, w, s) for c, w, s in terms if w > 0]
    neg = [(c, w, s) for c, w, s in terms if w < 0]
    ordered = pos + neg
    used_channels = sorted({c for c, _, _ in terms})
    cidx = {c: i for i, c in enumerate(used_channels)}
    nch = len(used_channels)
    nt = len(ordered)

    # Structure: one combined in-DMA per chunk, all wide-tile slices written
    # by ACT only, a single strided tensor_reduce per chunk, per-chunk clips
    # into slices of one wide result tile, and a single 1 MiB out-DMA (best
    # DMA shape, and keeps the sem-lane count low).
    with tile.TileContext(nc) as tc:
        with (
            tc.tile_pool(name="xin", bufs=N_CHUNKS) as xpool,
            tc.tile_pool(name="work", bufs=N_CHUNKS) as wpool,
            tc.tile_pool(name="out", bufs=min(2, max(1, reps))) as opool,
        ):
          for _ in range(reps):
            res_w = opool.tile([P, FREE], F32, tag="res")
            for j in range(N_CHUNKS):
                res = res_w[:, bass.ts(j, CHUNK)]
                if nt == 0:
                    nc.vector.memset(res, float(np.clip(bias, 0.0, 1.0)))
                    continue

                xt = xpool.tile([P, nch * CHUNK], F32, tag="x")
                src = x_t[used_channels[0]:used_channels[-1] + 1, :,
                          bass.ts(j, CHUNK)] if nch == C_IN else None
                if src is not None:
                    nc.sync.dma_start(
                        out=xt[:], in_=src.rearrange("c p f -> p c f")
                    )
                else:
                    for c in used_channels:
                        nc.sync.dma_start(
                            out=xt[:, bass.ts(cidx[c], CHUNK)],
                            in_=x_t[c, :, bass.ts(j, CHUNK)],
                        )

                wide = wpool.tile([P, nt * CHUNK], F32, tag="wide")
                for i, (c, w, s) in enumerate(ordered):
                    sl = wide[:, bass.ts(i, CHUNK)]
                    xs = xt[:, bass.ts(cidx[c], CHUNK)]
                    if w > 0:
                        nc.scalar.activation(
                            sl, xs, mybir.ActivationFunctionType.Relu,
                            bias=-w * s, scale=w,
                        )
                    else:
                        nc.scalar.activation(
                            sl, xs, mybir.ActivationFunctionType.Relu,
                            bias=w * s, scale=-w,
                        )

                def reduce_slices(lo, hi, tag):
                    n = hi - lo
                    dst = wpool.tile([P, CHUNK], F32, tag=tag)
                    if n == 1:
                        return wide[:, bass.ts(lo, CHUNK)]
                    v = wide[:, lo * CHUNK:hi * CHUNK].rearrange(
                        "p (c f) -> p f c", c=n
                    )
                    nc.vector.tensor_reduce(
                        dst[:], v, axis=mybir.AxisListType.X,
                        op=mybir.AluOpType.add,
                    )
                    return dst[:]

                if neg and pos:
                    rp = reduce_slices(0, len(pos), "redp")
                    rn = reduce_slices(len(pos), nt, "redn")
                    comb = wpool.tile([P, CHUNK], F32, tag="comb")
                    nc.vector.tensor_sub(comb[:], rp, rn)
                    comb = comb[:]
                elif pos:
                    comb = reduce_slices(0, len(pos), "redp")
                else:
                    rn = reduce_slices(0, nt, "redn")
                    comb = wpool.tile([P, CHUNK], F32, tag="comb")
                    nc.vector.tensor_scalar_mul(comb[:], rn, -1.0)
                    comb = comb[:]

                if bias != 0.0:
                    nc.vector.tensor_scalar(
                        res, comb, bias, 0.0,
                        mybir.AluOpType.add, mybir.AluOpType.max,
                    )
                    nc.vector.tensor_scalar_min(res, res, 1.0)
                else:
                    nc.vector.tensor_scalar(
                        res, comb, 0.0, 1.0,
                        mybir.AluOpType.max, mybir.AluOpType.min,
                    )
            nc.sync.dma_start(out=out_t[:, :], in_=res_w[:])
    nc.compile()
    return nc


_NC_CACHE = {}


def kernel(x, shift, slopes, conv_w, conv_b):
    global LAST_RESULTS
    x = np.ascontiguousarray(np.asarray(x, dtype=np.float32))
    shift = np.asarray(shift, dtype=np.float32)
    slopes = np.asarray(slopes, dtype=np.float32)
    conv_w = np.asarray(conv_w, dtype=np.float32)
    conv_b = np.asarray(conv_b, dtype=np.float32)

    B = x.shape[0]
    assert x.shape == (N_CORES, C_IN, H, W_IMG), x.shape

    wmat = slopes * conv_w[None, :]                      # (npts, C)
    npts = wmat.shape[0]
    terms = tuple(
        (c, float(wmat[p, c]), float(shift[p, c]))
        for p in range(npts) for c in range(C_IN)
        if wmat[p, c] != 0.0
    )
    bias = float(conv_b.reshape(-1)[0])

    key = (terms, bias)
    nc = _NC_CACHE.get(key)
    if nc is None:
        nc = _build_nc(terms, bias)
        _NC_CACHE[key] = nc

    xs = x.reshape(B, C_IN, P, FREE)
    in_maps = [{"x": xs[i]} for i in range(N_CORES)]
    trace = bool(int(os.environ.get("KERNEL_TRACE", "0")))
    LAST_RESULTS = run_bass_kernel_spmd(
        nc, in_maps, list(range(N_CORES)), trace=trace
    )
    out = np.stack(
        [LAST_RESULTS.results[i]["out"].reshape(1, H, W_IMG) for i in range(N_CORES)],
        axis=0,
    )
    return out.astype(np.float32, copy=False)
